# revision 33
# baseline (speedup 1.0000x reference)
"""Trainium2 Bass kernel for nn_ActionVectorQuantizer.

Vector quantizer: for each row of z [262144, 256], find the nearest of 4
codebook rows e_k (squared-L2 argmin), output (z_q = e[idx], idx).

Sharding: data-parallel over the batch across 8 NeuronCores; the [4, 256]
codebook is replicated.  Each core handles 32768 rows with no communication.

Math per row r (all on-device, fp32):
    argmin_k ||z_r - e_k||^2  ==  argmax_k h_k   where
    h_0 = 0,  h_k = 2<z_r, e_k - e_0> - (||e_k||^2 - ||e_0||^2)   k = 1..3
The argmax uses the DVE max/max_index ops (first-index tie-break, matching
jnp.argmin).  The three dot products per row are computed on two paths to
balance engines: DVE scalar_tensor_tensor with free-axis accumulate, or
PE (transpose z chunks through the tensor engine, then a [128x128]x[128x3]
fp32 matmul).  z_q is produced by a one-hot [4,128]x[4,256] matmul with a
split-bf16 codebook (exact to ~1e-6); PSUM->SBUF copies run on the scalar
engine.  idx is written partition-major and transposed on the host.
"""

import numpy as np

N_CORES = 8
BATCH = 262144
SHARD = BATCH // N_CORES          # 32768 rows per core
D = 256                           # code dim
K = 4                             # number of codes
P = 128                           # rows per tile (SBUF partitions)
TILES = SHARD // P                # 256 tiles per core
TB = 8                            # tiles per DMA batch (1 MiB per transfer)
NB = TILES // TB                  # 32 batches
NEG_BIG = -3.0e38

# Every PE_EVERY-th batch computes its dot products on the tensor engine
# instead of the DVE (engine balancing).  PE_EVERY = 0 disables the PE path.
PE_EVERY = 2


def build_nc(nb=NB, pe_every=PE_EVERY):
    import concourse.bass as bass
    import concourse.bacc as bacc
    import concourse.mybir as mybir
    from concourse.tile import TileContext
    from concourse.masks import make_identity
    from contextlib import ExitStack

    f32 = mybir.dt.float32
    f32r = mybir.dt.float32r
    bf16 = mybir.dt.bfloat16
    u32 = mybir.dt.uint32

    shard = nb * TB * P
    nc = bacc.Bacc()
    z = nc.declare_dram_parameter("z", [shard, D], f32, isOutput=False)
    emb = nc.declare_dram_parameter("embedding", [K, D], f32, isOutput=False)
    zq = nc.declare_dram_parameter("z_q", [shard, D], f32, isOutput=True)
    # partition-major: idx_pt[p, t] = argmin of row t*128 + p
    idx = nc.declare_dram_parameter("idx", [P, nb * TB], u32, isOutput=True)

    z_v = z.rearrange("(n t p) d -> n p t d", t=TB, p=P)        # [NB, 128, TB, D]
    zq_v = zq.rearrange("(n t p) d -> n p t d", t=TB, p=P)
    idx_v = idx.rearrange("p (n t) -> n p t", t=TB)             # [NB, 128, TB]

    with TileContext(nc) as tc, ExitStack() as ctx:
        const = ctx.enter_context(tc.tile_pool(name="const", bufs=1))
        io = ctx.enter_context(tc.tile_pool(name="io", bufs=3))
        work = ctx.enter_context(tc.tile_pool(name="work", bufs=2))
        psum = ctx.enter_context(tc.tile_pool(name="psum", bufs=2, space="PSUM"))

        # ---- constants -------------------------------------------------
        e_sb = const.tile([K, D], f32)
        nc.sync.dma_start(out=e_sb, in_=emb[:, :])

        ident = const.tile([P, P], bf16)
        make_identity(nc, ident)
        ident32 = const.tile([P, P], f32)
        make_identity(nc, ident32)

        # e rows broadcast to all 128 partitions (for the DVE dot path)
        e_bc = []
        for k in range(K):
            row = const.tile([1, D], f32, name=f"e_row{k}")
            nc.sync.dma_start(out=row, in_=emb[k : k + 1, :])
            t = const.tile([P, D], f32, name=f"e_bc{k}")
            nc.gpsimd.partition_broadcast(t, row)
            e_bc.append(t)

        # f_k = e_k - e_0 broadcast (k=1..3), DVE path operand
        f_bc = []
        for k in range(1, K):
            t = const.tile([P, D], f32, name=f"f_bc{k}")
            nc.vector.tensor_sub(t, e_bc[k], e_bc[0])
            f_bc.append(t)

        # ee[:, k] = ||e_k||^2 ; g_rep = (ee_k - ee_0) repeated TB times
        ee = const.tile([P, K], f32)
        scr_act = const.tile([P, D], f32)
        for k in range(K):
            nc.scalar.activation(
                out=scr_act,
                in_=e_bc[k],
                func=mybir.ActivationFunctionType.Square,
                accum_out=ee[:, k : k + 1],
            )
        gg = const.tile([P, K - 1], f32)
        nc.vector.tensor_scalar(
            gg, ee[:, 1:K], ee[:, 0:1], None, op0=mybir.AluOpType.subtract
        )
        g_rep = const.tile([P, (K - 1) * TB], f32)
        for t in range(TB):
            nc.vector.tensor_copy(g_rep[:, 3 * t : 3 * t + 3], gg)
        g_rep_v = g_rep.rearrange("p (t e) -> p t e", e=3)

        # fT chunks for the PE dot path: fT[:, 3c+j-1] = 2*(e_j - e_0)[c*128 + :]
        eT_ps = psum.tile([P, 3 * TB], f32, tag="s_ps")
        for c in range(2):
            nc.tensor.matmul(
                eT_ps[:, 4 * c : 4 * c + 4],
                e_sb[:, 128 * c : 128 * (c + 1)],
                ident32[0:K, 0:K],
                is_transpose=True,
                start=(c == 0),
                stop=(c == 1),
            )
        eT = const.tile([P, 2 * K], f32)
        nc.vector.tensor_copy(eT, eT_ps[:, 0 : 2 * K])
        fT = const.tile([P, 2 * (K - 1)], f32)
        for c in range(2):
            nc.vector.tensor_scalar(
                fT[:, 3 * c : 3 * c + 3],
                eT[:, 4 * c + 1 : 4 * c + 4],
                eT[:, 4 * c : 4 * c + 1],
                None,
                op0=mybir.AluOpType.subtract,
            )
        nc.vector.tensor_scalar_mul(fT, fT, 2.0)

        # split-bf16 codebook for the exact one-hot gather
        e_hi = const.tile([K, D], bf16)
        nc.vector.tensor_copy(e_hi, e_sb)
        e_hi_f = const.tile([K, D], f32)
        nc.vector.tensor_copy(e_hi_f, e_hi)
        e_lo_f = const.tile([K, D], f32)
        nc.vector.tensor_sub(e_lo_f, e_sb, e_hi_f)
        e_lo = const.tile([K, D], bf16)
        nc.vector.tensor_copy(e_lo, e_lo_f)

        # iota row 0..3 along free (for the one-hot compare)
        iota_t = const.tile([P, K], f32)
        for k in range(K):
            nc.vector.memset(iota_t[:, k : k + 1], float(k))

        # ---- main loop -------------------------------------------------
        for b in range(nb):
            pe_path = pe_every > 0 and (b % pe_every == 0)

            z_b = io.tile([P, TB * D], f32, tag="z_b")
            nc.sync.dma_start(out=z_b, in_=z_v[b])
            zq_b = io.tile([P, TB * D], f32, tag="zq_b")

            h_b = work.tile([P, 8 * TB], f32, tag="h_b")
            nc.vector.memset(h_b, NEG_BIG)
            h_v = h_b.rearrange("p (t e) -> p t e", e=8)
            nc.vector.memset(h_v[:, :, 0:1], 0.0)

            vt = work.tile([P, 8 * TB], u32, tag="vt")
            scr = work.tile([P, D], f32, tag="ttr_scr")
            mx = work.tile([P, 8], f32, tag="mx")

            # --- h_k = 2<z, e_k - e_0> - (ee_k - ee_0), k = 1..3 -------
            if pe_path:
                s_ps = psum.tile([P, 3 * TB], f32, tag="s_ps")
                for t in range(TB):
                    tp_ps = psum.tile([P, D], f32, tag="tp_ps")
                    for c in range(2):
                        nc.tensor.matmul(
                            tp_ps[:, 128 * c : 128 * (c + 1)],
                            z_b[:, t * D + 128 * c : t * D + 128 * (c + 1)],
                            ident32,
                            is_transpose=True,
                            start=(c == 0),
                            stop=(c == 1),
                        )
                    zT = work.tile([P, D], f32, tag="zT", bufs=3)
                    nc.scalar.copy(out=zT, in_=tp_ps)
                    for c in range(2):
                        nc.tensor.matmul(
                            s_ps[:, 3 * t : 3 * t + 3],
                            zT[:, 128 * c : 128 * (c + 1)],
                            fT[:, 3 * c : 3 * c + 3],
                            start=(t == 0 and c == 0),
                            stop=(t == TB - 1 and c == 1),
                            skip_group_check=not (
                                (t == 0 and c == 0) or (t == TB - 1 and c == 1)
                            ),
                        )
                s_v = s_ps.rearrange("p (t e) -> p t e", e=3)
                nc.vector.tensor_tensor(
                    out=h_v[:, :, 1:4],
                    in0=s_v,
                    in1=g_rep_v,
                    op=mybir.AluOpType.subtract,
                )
            else:
                for t in range(TB):
                    zt = z_b[:, t * D : (t + 1) * D]
                    hs = h_b[:, 8 * t : 8 * t + 8]
                    for k in range(1, K):
                        nc.vector.scalar_tensor_tensor(
                            out=scr,
                            in0=zt,
                            scalar=2.0,
                            in1=f_bc[k - 1],
                            op0=mybir.AluOpType.mult,
                            op1=mybir.AluOpType.mult,
                            accum_out=hs[:, k : k + 1],
                        )
                nc.vector.tensor_tensor(
                    out=h_v[:, :, 1:4],
                    in0=h_v[:, :, 1:4],
                    in1=g_rep_v,
                    op=mybir.AluOpType.subtract,
                )

            # --- argmax + one-hot + gather -----------------------------
            for t in range(TB):
                hs = h_b[:, 8 * t : 8 * t + 8]
                nc.vector.max(out=mx, in_=hs)
                nc.vector.max_index(
                    out=vt[:, 8 * t : 8 * t + 8], in_max=mx, in_values=hs
                )
            idxf = work.tile([P, TB], f32, tag="idxf")
            nc.vector.tensor_copy(idxf, vt[:, 0 : 8 * TB : 8])

            for q in range(TB // 4):
                ot_ps = psum.tile([K, 4 * P], bf16, tag="ot_ps")
                for u in range(4):
                    t = 4 * q + u
                    O_t = work.tile([P, K], bf16, tag="O_t", bufs=3)
                    nc.vector.tensor_scalar(
                        O_t,
                        iota_t,
                        idxf[:, t : t + 1],
                        None,
                        op0=mybir.AluOpType.is_equal,
                    )
                    nc.tensor.matmul(
                        ot_ps[:, P * u : P * (u + 1)],
                        O_t,
                        ident,
                        is_transpose=True,
                        start=(u == 0),
                        stop=(u == 3),
                        skip_group_check=not (u == 0 or u == 3),
                    )
                ot_sb = work.tile([K, 4 * P], bf16, tag="ot_sb")
                nc.scalar.copy(out=ot_sb, in_=ot_ps)

                for u in range(0, 4, 2):
                    zq_ps = psum.tile([P, 2 * D], f32, tag="zq_ps")
                    for v in range(2):
                        lhsT = ot_sb[:, P * (u + v) : P * (u + v + 1)]
                        nc.tensor.matmul(
                            zq_ps[:, v * D : (v + 1) * D],
                            lhsT,
                            e_hi,
                            start=(v == 0),
                            stop=False,
                            skip_group_check=(v != 0),
                        )
                        nc.tensor.matmul(
                            zq_ps[:, v * D : (v + 1) * D],
                            lhsT,
                            e_lo,
                            start=False,
                            stop=(v == 1),
                            skip_group_check=(v != 1),
                        )
                    t0 = 4 * q + u
                    nc.scalar.copy(
                        out=zq_b[:, t0 * D : (t0 + 2) * D], in_=zq_ps
                    )

            nc.sync.dma_start(out=zq_v[b], in_=zq_b)
            nc.scalar.dma_start(out=idx_v[b], in_=vt[:, 0 : 8 * TB : 8])

    nc.compile()
    return nc


_NC = None


def _get_nc():
    global _NC
    if _NC is None:
        _NC = build_nc()
    return _NC


def kernel(**inputs):
    from concourse.bass_utils import run_bass_kernel_spmd

    z = np.ascontiguousarray(np.asarray(inputs["z"], dtype=np.float32))
    emb = np.ascontiguousarray(np.asarray(inputs["embedding"], dtype=np.float32))
    nc = _get_nc()
    in_maps = [
        {"z": z[i * SHARD : (i + 1) * SHARD], "embedding": emb}
        for i in range(N_CORES)
    ]
    res = run_bass_kernel_spmd(nc, in_maps, list(range(N_CORES))).results
    z_q = np.concatenate([r["z_q"] for r in res], axis=0)
    idx = np.concatenate(
        [r["idx"].T.reshape(-1) for r in res], axis=0
    ).astype(np.int32)
    return z_q, idx


# revision 34
# speedup vs baseline: 1.0105x; 1.0105x over previous
"""Trainium2 Bass kernel for nn_ActionVectorQuantizer.

Vector quantizer: for each row of z [262144, 256], find the nearest of 4
codebook rows e_k (squared-L2 argmin), output (z_q = e[idx], idx).

Sharding: data-parallel over the batch across 8 NeuronCores; the [4, 256]
codebook is replicated.  Each core handles 32768 rows with no communication.

Math per row r (all on-device, fp32):
    argmin_k ||z_r - e_k||^2  ==  argmax_k h_k   where
    h_0 = 0,  h_k = 2<z_r, e_k - e_0> - (||e_k||^2 - ||e_0||^2)   k = 1..3
The argmax uses the DVE max/max_index ops (first-index tie-break, matching
jnp.argmin).  The three dot products per row are computed on two paths to
balance engines: DVE scalar_tensor_tensor with free-axis accumulate, or
PE (transpose z chunks through the tensor engine, then a [128x128]x[128x3]
fp32 matmul).  z_q is produced by a one-hot [4,128]x[4,256] matmul with a
split-bf16 codebook (exact to ~1e-6); PSUM->SBUF copies run on the scalar
engine.  idx is written partition-major and transposed on the host.
"""

import numpy as np

N_CORES = 8
BATCH = 262144
SHARD = BATCH // N_CORES          # 32768 rows per core
D = 256                           # code dim
K = 4                             # number of codes
P = 128                           # rows per tile (SBUF partitions)
TILES = SHARD // P                # 256 tiles per core
TB = 8                            # tiles per DMA batch (1 MiB per transfer)
NB = TILES // TB                  # 32 batches
NEG_BIG = -3.0e38

# Every PE_EVERY-th batch computes its dot products on the tensor engine
# instead of the DVE (engine balancing).  PE_EVERY = 0 disables the PE path.
PE_EVERY = 0


def build_nc(nb=NB, pe_every=PE_EVERY):
    import concourse.bass as bass
    import concourse.bacc as bacc
    import concourse.mybir as mybir
    from concourse.tile import TileContext
    from concourse.masks import make_identity
    from contextlib import ExitStack

    f32 = mybir.dt.float32
    f32r = mybir.dt.float32r
    bf16 = mybir.dt.bfloat16
    u32 = mybir.dt.uint32

    shard = nb * TB * P
    nc = bacc.Bacc()
    z = nc.declare_dram_parameter("z", [shard, D], f32, isOutput=False)
    emb = nc.declare_dram_parameter("embedding", [K, D], f32, isOutput=False)
    zq = nc.declare_dram_parameter("z_q", [shard, D], f32, isOutput=True)
    # partition-major: idx_pt[p, t] = argmin of row t*128 + p
    idx = nc.declare_dram_parameter("idx", [P, nb * TB], u32, isOutput=True)

    z_v = z.rearrange("(n t p) d -> n p t d", t=TB, p=P)        # [NB, 128, TB, D]
    zq_v = zq.rearrange("(n t p) d -> n p t d", t=TB, p=P)
    idx_v = idx.rearrange("p (n t) -> n p t", t=TB)             # [NB, 128, TB]

    with TileContext(nc) as tc, ExitStack() as ctx:
        const = ctx.enter_context(tc.tile_pool(name="const", bufs=1))
        io = ctx.enter_context(tc.tile_pool(name="io", bufs=3))
        work = ctx.enter_context(tc.tile_pool(name="work", bufs=2))
        psum = ctx.enter_context(tc.tile_pool(name="psum", bufs=2, space="PSUM"))

        # ---- constants -------------------------------------------------
        e_sb = const.tile([K, D], f32)
        nc.sync.dma_start(out=e_sb, in_=emb[:, :])

        ident = const.tile([P, P], bf16)
        make_identity(nc, ident)
        ident32 = const.tile([P, P], f32)
        make_identity(nc, ident32)

        # e rows broadcast to all 128 partitions (for the DVE dot path)
        e_bc = []
        for k in range(K):
            row = const.tile([1, D], f32, name=f"e_row{k}")
            nc.sync.dma_start(out=row, in_=emb[k : k + 1, :])
            t = const.tile([P, D], f32, name=f"e_bc{k}")
            nc.gpsimd.partition_broadcast(t, row)
            e_bc.append(t)

        # f_k = e_k - e_0 broadcast (k=1..3), DVE path operand
        f_bc = []
        for k in range(1, K):
            t = const.tile([P, D], f32, name=f"f_bc{k}")
            nc.vector.tensor_sub(t, e_bc[k], e_bc[0])
            f_bc.append(t)

        # ee[:, k] = ||e_k||^2 ; g_rep = (ee_k - ee_0) repeated TB times
        ee = const.tile([P, K], f32)
        scr_act = const.tile([P, D], f32)
        for k in range(K):
            nc.scalar.activation(
                out=scr_act,
                in_=e_bc[k],
                func=mybir.ActivationFunctionType.Square,
                accum_out=ee[:, k : k + 1],
            )
        gg = const.tile([P, K - 1], f32)
        nc.vector.tensor_scalar(
            gg, ee[:, 1:K], ee[:, 0:1], None, op0=mybir.AluOpType.subtract
        )
        g_rep = const.tile([P, (K - 1) * TB], f32)
        for t in range(TB):
            nc.vector.tensor_copy(g_rep[:, 3 * t : 3 * t + 3], gg)
        g_rep_v = g_rep.rearrange("p (t e) -> p t e", e=3)

        # fT chunks for the PE dot path: fT[:, 3c+j-1] = 2*(e_j - e_0)[c*128 + :]
        eT_ps = psum.tile([P, 3 * TB], f32, tag="s_ps")
        for c in range(2):
            nc.tensor.matmul(
                eT_ps[:, 4 * c : 4 * c + 4],
                e_sb[:, 128 * c : 128 * (c + 1)],
                ident32[0:K, 0:K],
                is_transpose=True,
                start=(c == 0),
                stop=(c == 1),
            )
        eT = const.tile([P, 2 * K], f32)
        nc.vector.tensor_copy(eT, eT_ps[:, 0 : 2 * K])
        fT = const.tile([P, 2 * (K - 1)], f32)
        for c in range(2):
            nc.vector.tensor_scalar(
                fT[:, 3 * c : 3 * c + 3],
                eT[:, 4 * c + 1 : 4 * c + 4],
                eT[:, 4 * c : 4 * c + 1],
                None,
                op0=mybir.AluOpType.subtract,
            )
        nc.vector.tensor_scalar_mul(fT, fT, 2.0)

        # split-bf16 codebook for the exact one-hot gather
        e_hi = const.tile([K, D], bf16)
        nc.vector.tensor_copy(e_hi, e_sb)
        e_hi_f = const.tile([K, D], f32)
        nc.vector.tensor_copy(e_hi_f, e_hi)
        e_lo_f = const.tile([K, D], f32)
        nc.vector.tensor_sub(e_lo_f, e_sb, e_hi_f)
        e_lo = const.tile([K, D], bf16)
        nc.vector.tensor_copy(e_lo, e_lo_f)

        # iota row 0..3 along free, repeated TB times (one-hot compare)
        iota_rep = const.tile([P, K * TB], f32)
        for k in range(K):
            nc.vector.memset(
                iota_rep.rearrange("p (t e) -> p t e", e=K)[:, :, k : k + 1],
                float(k),
            )

        # ---- main loop -------------------------------------------------
        for b in range(nb):
            pe_path = pe_every > 0 and (b % pe_every == 0)

            z_b = io.tile([P, TB * D], f32, tag="z_b")
            nc.sync.dma_start(out=z_b, in_=z_v[b])
            zq_b = io.tile([P, TB * D], f32, tag="zq_b")

            h_b = work.tile([P, 8 * TB], f32, tag="h_b")
            nc.gpsimd.memset(h_b, NEG_BIG)
            h_v = h_b.rearrange("p (t e) -> p t e", e=8)
            nc.gpsimd.memset(h_v[:, :, 0:1], 0.0)

            vt = work.tile([P, 8 * TB], u32, tag="vt")
            scr = work.tile([P, D], f32, tag="ttr_scr")
            mx = work.tile([P, 8], f32, tag="mx")

            # --- h_k = 2<z, e_k - e_0> - (ee_k - ee_0), k = 1..3 -------
            if pe_path:
                s_ps = psum.tile([P, 3 * TB], f32, tag="s_ps")
                for t in range(TB):
                    tp_ps = psum.tile([P, D], f32, tag="tp_ps")
                    for c in range(2):
                        nc.tensor.matmul(
                            tp_ps[:, 128 * c : 128 * (c + 1)],
                            z_b[:, t * D + 128 * c : t * D + 128 * (c + 1)],
                            ident32,
                            is_transpose=True,
                            start=(c == 0),
                            stop=(c == 1),
                        )
                    zT = work.tile([P, D], f32, tag="zT", bufs=3)
                    nc.scalar.copy(out=zT, in_=tp_ps)
                    for c in range(2):
                        nc.tensor.matmul(
                            s_ps[:, 3 * t : 3 * t + 3],
                            zT[:, 128 * c : 128 * (c + 1)],
                            fT[:, 3 * c : 3 * c + 3],
                            start=(t == 0 and c == 0),
                            stop=(t == TB - 1 and c == 1),
                            skip_group_check=not (
                                (t == 0 and c == 0) or (t == TB - 1 and c == 1)
                            ),
                        )
                s_v = s_ps.rearrange("p (t e) -> p t e", e=3)
                nc.vector.tensor_tensor(
                    out=h_v[:, :, 1:4],
                    in0=s_v,
                    in1=g_rep_v,
                    op=mybir.AluOpType.subtract,
                )
            else:
                for t in range(TB):
                    zt = z_b[:, t * D : (t + 1) * D]
                    hs = h_b[:, 8 * t : 8 * t + 8]
                    for k in range(1, K):
                        nc.vector.scalar_tensor_tensor(
                            out=scr,
                            in0=zt,
                            scalar=2.0,
                            in1=f_bc[k - 1],
                            op0=mybir.AluOpType.mult,
                            op1=mybir.AluOpType.mult,
                            accum_out=hs[:, k : k + 1],
                        )
                nc.vector.tensor_tensor(
                    out=h_v[:, :, 1:4],
                    in0=h_v[:, :, 1:4],
                    in1=g_rep_v,
                    op=mybir.AluOpType.subtract,
                )

            # --- argmax + one-hot + gather -----------------------------
            for t in range(TB):
                hs = h_b[:, 8 * t : 8 * t + 8]
                nc.vector.max(out=mx, in_=hs)
                nc.vector.max_index(
                    out=vt[:, 8 * t : 8 * t + 8], in_max=mx, in_values=hs
                )
            idxf = work.tile([P, TB], f32, tag="idxf")
            nc.vector.tensor_copy(idxf, vt[:, 0 : 8 * TB : 8])
            O_big = work.tile([P, K * TB], bf16, tag="O_big")
            nc.vector.tensor_tensor(
                out=O_big.rearrange("p (t e) -> p t e", e=K),
                in0=iota_rep.rearrange("p (t e) -> p t e", e=K),
                in1=idxf[:, :].to_broadcast([P, TB, K]),
                op=mybir.AluOpType.is_equal,
            )

            for q in range(TB // 4):
                ot_ps = psum.tile([K, 4 * P], bf16, tag="ot_ps")
                for u in range(4):
                    t = 4 * q + u
                    nc.tensor.matmul(
                        ot_ps[:, P * u : P * (u + 1)],
                        O_big[:, K * t : K * (t + 1)],
                        ident,
                        is_transpose=True,
                        start=(u == 0),
                        stop=(u == 3),
                        skip_group_check=not (u == 0 or u == 3),
                    )
                ot_sb = work.tile([K, 4 * P], bf16, tag="ot_sb")
                nc.scalar.copy(out=ot_sb, in_=ot_ps)

                for u in range(0, 4, 2):
                    zq_ps = psum.tile([P, 2 * D], f32, tag="zq_ps")
                    for v in range(2):
                        lhsT = ot_sb[:, P * (u + v) : P * (u + v + 1)]
                        nc.tensor.matmul(
                            zq_ps[:, v * D : (v + 1) * D],
                            lhsT,
                            e_hi,
                            start=(v == 0),
                            stop=False,
                            skip_group_check=(v != 0),
                        )
                        nc.tensor.matmul(
                            zq_ps[:, v * D : (v + 1) * D],
                            lhsT,
                            e_lo,
                            start=False,
                            stop=(v == 1),
                            skip_group_check=(v != 1),
                        )
                    t0 = 4 * q + u
                    nc.scalar.copy(
                        out=zq_b[:, t0 * D : (t0 + 2) * D], in_=zq_ps
                    )

            nc.sync.dma_start(out=zq_v[b], in_=zq_b)
            nc.scalar.dma_start(out=idx_v[b], in_=vt[:, 0 : 8 * TB : 8])

    nc.compile()
    return nc


_NC = None


def _get_nc():
    global _NC
    if _NC is None:
        _NC = build_nc()
    return _NC


def kernel(**inputs):
    from concourse.bass_utils import run_bass_kernel_spmd

    z = np.ascontiguousarray(np.asarray(inputs["z"], dtype=np.float32))
    emb = np.ascontiguousarray(np.asarray(inputs["embedding"], dtype=np.float32))
    nc = _get_nc()
    in_maps = [
        {"z": z[i * SHARD : (i + 1) * SHARD], "embedding": emb}
        for i in range(N_CORES)
    ]
    res = run_bass_kernel_spmd(nc, in_maps, list(range(N_CORES))).results
    z_q = np.concatenate([r["z_q"] for r in res], axis=0)
    idx = np.concatenate(
        [r["idx"].T.reshape(-1) for r in res], axis=0
    ).astype(np.int32)
    return z_q, idx


# revision 35
# speedup vs baseline: 1.2558x; 1.2428x over previous
"""Trainium2 Bass kernel for nn_ActionVectorQuantizer.

Vector quantizer: for each row of z [262144, 256], find the nearest of 4
codebook rows e_k (squared-L2 argmin), output (z_q = e[idx], idx).

Sharding: data-parallel over the batch across 8 NeuronCores; the [4, 256]
codebook is replicated.  Each core handles 32768 rows with no communication.

Math per row r (all on-device, fp32):
    argmin_k ||z_r - e_k||^2  ==  argmax_k h_k   where
    h_0 = 0,  h_k = 2<z_r, e_k - e_0> - (||e_k||^2 - ||e_0||^2)   k = 1..3
The argmax uses the DVE max/max_index ops (first-index tie-break, matching
jnp.argmin).  The three dot products per row are computed on two paths to
balance engines: DVE scalar_tensor_tensor with free-axis accumulate, or
PE (transpose z chunks through the tensor engine, then a [128x128]x[128x3]
fp32 matmul).  z_q is produced by a one-hot [4,128]x[4,256] matmul with a
split-bf16 codebook (exact to ~1e-6); PSUM->SBUF copies run on the scalar
engine.  idx is written partition-major and transposed on the host.
"""

import numpy as np

N_CORES = 8
BATCH = 262144
SHARD = BATCH // N_CORES          # 32768 rows per core
D = 256                           # code dim
K = 4                             # number of codes
P = 128                           # rows per tile (SBUF partitions)
TILES = SHARD // P                # 256 tiles per core
TB = 8                            # tiles per DMA batch (1 MiB per transfer)
NB = TILES // TB                  # 32 batches
NEG_BIG = -3.0e38

# Every PE_EVERY-th batch computes its dot products on the tensor engine
# instead of the DVE (engine balancing).  PE_EVERY = 0 disables the PE path.
PE_EVERY = 0


def build_nc(nb=NB, pe_every=PE_EVERY):
    import concourse.bass as bass
    import concourse.bacc as bacc
    import concourse.mybir as mybir
    from concourse.tile import TileContext
    from concourse.masks import make_identity
    from contextlib import ExitStack

    f32 = mybir.dt.float32
    f32r = mybir.dt.float32r
    bf16 = mybir.dt.bfloat16
    u32 = mybir.dt.uint32

    shard = nb * TB * P
    nc = bacc.Bacc()
    z = nc.declare_dram_parameter("z", [shard, D], f32, isOutput=False)
    emb = nc.declare_dram_parameter("embedding", [K, D], f32, isOutput=False)
    zq = nc.declare_dram_parameter("z_q", [shard, D], f32, isOutput=True)
    # partition-major: idx_pt[p, t] = argmin of row t*128 + p
    idx = nc.declare_dram_parameter("idx", [P, nb * TB], u32, isOutput=True)

    z_v = z.rearrange("(n t p) d -> n p t d", t=TB, p=P)        # [NB, 128, TB, D]
    zq_v = zq.rearrange("(n t p) d -> n p t d", t=TB, p=P)
    idx_v = idx.rearrange("p (n t) -> n p t", t=TB)             # [NB, 128, TB]

    with TileContext(nc) as tc, ExitStack() as ctx:
        const = ctx.enter_context(tc.tile_pool(name="const", bufs=1))
        io = ctx.enter_context(tc.tile_pool(name="io", bufs=4))
        work = ctx.enter_context(tc.tile_pool(name="work", bufs=3))
        psum = ctx.enter_context(tc.tile_pool(name="psum", bufs=2, space="PSUM"))

        # ---- constants -------------------------------------------------
        e_sb = const.tile([K, D], f32)
        nc.sync.dma_start(out=e_sb, in_=emb[:, :])

        ident = const.tile([P, P], bf16)
        make_identity(nc, ident)
        ident32 = const.tile([P, P], f32)
        make_identity(nc, ident32)

        # e rows broadcast to all 128 partitions (for the DVE dot path)
        e_bc = []
        for k in range(K):
            row = const.tile([1, D], f32, name=f"e_row{k}")
            nc.sync.dma_start(out=row, in_=emb[k : k + 1, :])
            t = const.tile([P, D], f32, name=f"e_bc{k}")
            nc.gpsimd.partition_broadcast(t, row)
            e_bc.append(t)

        # f_k = e_k - e_0 broadcast (k=1..3), DVE path operand
        f_bc = []
        for k in range(1, K):
            t = const.tile([P, D], f32, name=f"f_bc{k}")
            nc.vector.tensor_sub(t, e_bc[k], e_bc[0])
            f_bc.append(t)

        # ee[:, k] = ||e_k||^2 ; g_rep = (ee_k - ee_0) repeated TB times
        ee = const.tile([P, K], f32)
        scr_act = const.tile([P, D], f32)
        for k in range(K):
            nc.scalar.activation(
                out=scr_act,
                in_=e_bc[k],
                func=mybir.ActivationFunctionType.Square,
                accum_out=ee[:, k : k + 1],
            )
        gg = const.tile([P, K - 1], f32)
        nc.vector.tensor_scalar(
            gg, ee[:, 1:K], ee[:, 0:1], None, op0=mybir.AluOpType.subtract
        )
        g_rep = const.tile([P, (K - 1) * TB], f32)
        for t in range(TB):
            nc.vector.tensor_copy(g_rep[:, 3 * t : 3 * t + 3], gg)
        g_rep_v = g_rep.rearrange("p (t e) -> p t e", e=3)

        # fT chunks for the PE dot path: fT[:, 3c+j-1] = 2*(e_j - e_0)[c*128 + :]
        eT_ps = psum.tile([P, 3 * TB], f32, tag="s_ps")
        for c in range(2):
            nc.tensor.matmul(
                eT_ps[:, 4 * c : 4 * c + 4],
                e_sb[:, 128 * c : 128 * (c + 1)],
                ident32[0:K, 0:K],
                is_transpose=True,
                start=(c == 0),
                stop=(c == 1),
            )
        eT = const.tile([P, 2 * K], f32)
        nc.vector.tensor_copy(eT, eT_ps[:, 0 : 2 * K])
        fT = const.tile([P, 2 * (K - 1)], f32)
        for c in range(2):
            nc.vector.tensor_scalar(
                fT[:, 3 * c : 3 * c + 3],
                eT[:, 4 * c + 1 : 4 * c + 4],
                eT[:, 4 * c : 4 * c + 1],
                None,
                op0=mybir.AluOpType.subtract,
            )
        nc.vector.tensor_scalar_mul(fT, fT, 2.0)

        # split-bf16 codebook for the exact one-hot gather
        e_hi = const.tile([K, D], bf16)
        nc.vector.tensor_copy(e_hi, e_sb)
        e_hi_f = const.tile([K, D], f32)
        nc.vector.tensor_copy(e_hi_f, e_hi)
        e_lo_f = const.tile([K, D], f32)
        nc.vector.tensor_sub(e_lo_f, e_sb, e_hi_f)
        e_lo = const.tile([K, D], bf16)
        nc.vector.tensor_copy(e_lo, e_lo_f)

        # iota row 0..3 along free, repeated TB times (one-hot compare)
        iota_rep = const.tile([P, K * TB], f32)
        for k in range(K):
            nc.vector.memset(
                iota_rep.rearrange("p (t e) -> p t e", e=K)[:, :, k : k + 1],
                float(k),
            )

        # ---- main loop -------------------------------------------------
        for b in range(nb):
            pe_path = pe_every > 0 and (b % pe_every == 0)

            z_b = io.tile([P, TB * D], f32, tag="z_b")
            nc.sync.dma_start(out=z_b, in_=z_v[b])
            zq_b = io.tile([P, TB * D], f32, tag="zq_b")

            h_b = work.tile([P, 8 * TB], f32, tag="h_b")
            nc.vector.memset(h_b, NEG_BIG)
            h_v = h_b.rearrange("p (t e) -> p t e", e=8)
            nc.vector.memset(h_v[:, :, 0:1], 0.0)

            vt = work.tile([P, 8 * TB], u32, tag="vt")
            scr = work.tile([P, D], f32, tag="ttr_scr")
            mx = work.tile([P, 8], f32, tag="mx")

            # --- h_k = 2<z, e_k - e_0> - (ee_k - ee_0), k = 1..3 -------
            if pe_path:
                s_ps = psum.tile([P, 3 * TB], f32, tag="s_ps")
                for t in range(TB):
                    tp_ps = psum.tile([P, D], f32, tag="tp_ps")
                    for c in range(2):
                        nc.tensor.matmul(
                            tp_ps[:, 128 * c : 128 * (c + 1)],
                            z_b[:, t * D + 128 * c : t * D + 128 * (c + 1)],
                            ident32,
                            is_transpose=True,
                            start=(c == 0),
                            stop=(c == 1),
                        )
                    zT = work.tile([P, D], f32, tag="zT", bufs=3)
                    nc.scalar.copy(out=zT, in_=tp_ps)
                    for c in range(2):
                        nc.tensor.matmul(
                            s_ps[:, 3 * t : 3 * t + 3],
                            zT[:, 128 * c : 128 * (c + 1)],
                            fT[:, 3 * c : 3 * c + 3],
                            start=(t == 0 and c == 0),
                            stop=(t == TB - 1 and c == 1),
                            skip_group_check=not (
                                (t == 0 and c == 0) or (t == TB - 1 and c == 1)
                            ),
                        )
                s_v = s_ps.rearrange("p (t e) -> p t e", e=3)
                nc.vector.tensor_tensor(
                    out=h_v[:, :, 1:4],
                    in0=s_v,
                    in1=g_rep_v,
                    op=mybir.AluOpType.subtract,
                )
            else:
                for t in range(TB):
                    zt = z_b[:, t * D : (t + 1) * D]
                    hs = h_b[:, 8 * t : 8 * t + 8]
                    for k in range(1, K):
                        nc.vector.scalar_tensor_tensor(
                            out=scr,
                            in0=zt,
                            scalar=2.0,
                            in1=f_bc[k - 1],
                            op0=mybir.AluOpType.mult,
                            op1=mybir.AluOpType.mult,
                            accum_out=hs[:, k : k + 1],
                        )
                nc.vector.tensor_tensor(
                    out=h_v[:, :, 1:4],
                    in0=h_v[:, :, 1:4],
                    in1=g_rep_v,
                    op=mybir.AluOpType.subtract,
                )

            # --- argmax + one-hot + gather -----------------------------
            for t in range(TB):
                hs = h_b[:, 8 * t : 8 * t + 8]
                nc.vector.max(out=mx, in_=hs)
                nc.vector.max_index(
                    out=vt[:, 8 * t : 8 * t + 8], in_max=mx, in_values=hs
                )
            idxf = work.tile([P, TB], f32, tag="idxf")
            nc.vector.tensor_copy(idxf, vt[:, 0 : 8 * TB : 8])
            O_big = work.tile([P, K * TB], bf16, tag="O_big")
            nc.vector.tensor_tensor(
                out=O_big.rearrange("p (t e) -> p t e", e=K),
                in0=iota_rep.rearrange("p (t e) -> p t e", e=K),
                in1=idxf[:, :].to_broadcast([P, TB, K]),
                op=mybir.AluOpType.is_equal,
            )

            for q in range(TB // 4):
                ot_ps = psum.tile([K, 4 * P], bf16, tag="ot_ps", bufs=3)
                for u in range(4):
                    t = 4 * q + u
                    nc.tensor.matmul(
                        ot_ps[:, P * u : P * (u + 1)],
                        O_big[:, K * t : K * (t + 1)],
                        ident,
                        is_transpose=True,
                        start=(u == 0),
                        stop=(u == 3),
                        skip_group_check=not (u == 0 or u == 3),
                    )
                ot_sb = work.tile([K, 4 * P], bf16, tag="ot_sb")
                nc.scalar.copy(out=ot_sb, in_=ot_ps)

                for u in range(0, 4, 2):
                    zq_ps = psum.tile([P, 2 * D], f32, tag="zq_ps", bufs=3)
                    for v in range(2):
                        lhsT = ot_sb[:, P * (u + v) : P * (u + v + 1)]
                        nc.tensor.matmul(
                            zq_ps[:, v * D : (v + 1) * D],
                            lhsT,
                            e_hi,
                            start=(v == 0),
                            stop=False,
                            skip_group_check=(v != 0),
                        )
                        nc.tensor.matmul(
                            zq_ps[:, v * D : (v + 1) * D],
                            lhsT,
                            e_lo,
                            start=False,
                            stop=(v == 1),
                            skip_group_check=(v != 1),
                        )
                    t0 = 4 * q + u
                    nc.scalar.copy(
                        out=zq_b[:, t0 * D : (t0 + 2) * D], in_=zq_ps
                    )

            nc.gpsimd.dma_start(out=zq_v[b], in_=zq_b)
            nc.scalar.dma_start(out=idx_v[b], in_=vt[:, 0 : 8 * TB : 8])

    nc.compile()
    return nc


_NC = None


def _get_nc():
    global _NC
    if _NC is None:
        _NC = build_nc()
    return _NC


def kernel(**inputs):
    from concourse.bass_utils import run_bass_kernel_spmd

    z = np.ascontiguousarray(np.asarray(inputs["z"], dtype=np.float32))
    emb = np.ascontiguousarray(np.asarray(inputs["embedding"], dtype=np.float32))
    nc = _get_nc()
    in_maps = [
        {"z": z[i * SHARD : (i + 1) * SHARD], "embedding": emb}
        for i in range(N_CORES)
    ]
    res = run_bass_kernel_spmd(nc, in_maps, list(range(N_CORES))).results
    z_q = np.concatenate([r["z_q"] for r in res], axis=0)
    idx = np.concatenate(
        [r["idx"].T.reshape(-1) for r in res], axis=0
    ).astype(np.int32)
    return z_q, idx


# revision 40
# speedup vs baseline: 1.9255x; 1.5332x over previous
"""Trainium2 Bass kernel for nn_ActionVectorQuantizer.

Vector quantizer: for each row of z [262144, 256], find the nearest of 4
codebook rows e_k (squared-L2 argmin), output (z_q = e[idx], idx).

Sharding: data-parallel over the batch across 8 NeuronCores; the [4, 256]
codebook is replicated.  Each core handles 32768 rows with no communication.

Math per row r (all on-device, fp32):
    argmin_k ||z_r - e_k||^2  ==  argmax_k h_k   where
    h_0 = 0,  h_k = 2<z_r, e_k - e_0> - (||e_k||^2 - ||e_0||^2)   k = 1..3
The argmax uses the DVE max/max_index ops (first-index tie-break, matching
jnp.argmin).  The three dot products per row are computed on two paths to
balance engines: DVE scalar_tensor_tensor with free-axis accumulate, or
PE (transpose z chunks through the tensor engine, then a [128x128]x[128x3]
fp32 matmul).  z_q is produced by a one-hot [4,128]x[4,256] matmul with a
split-bf16 codebook (exact to ~1e-6); PSUM->SBUF copies run on the scalar
engine.  idx is written partition-major and transposed on the host.
"""

import numpy as np

N_CORES = 8
BATCH = 262144
SHARD = BATCH // N_CORES          # 32768 rows per core
D = 256                           # code dim
K = 4                             # number of codes
P = 128                           # rows per tile (SBUF partitions)
TILES = SHARD // P                # 256 tiles per core
TB = 8                            # tiles per DMA batch (1 MiB per transfer)
NB = TILES // TB                  # 32 batches
NEG_BIG = -3.0e38

# Every PE_EVERY-th batch computes its dot products on the tensor engine
# instead of the DVE (engine balancing).  PE_EVERY = 0 disables the PE path.
PE_EVERY = 0


def build_nc(nb=NB, pe_every=PE_EVERY):
    import concourse.bass as bass
    import concourse.bacc as bacc
    import concourse.mybir as mybir
    from concourse.tile import TileContext
    from concourse.masks import make_identity
    from contextlib import ExitStack

    f32 = mybir.dt.float32
    f32r = mybir.dt.float32r
    bf16 = mybir.dt.bfloat16
    u32 = mybir.dt.uint32

    shard = nb * TB * P
    nc = bacc.Bacc()
    z = nc.declare_dram_parameter("z", [shard, D], f32, isOutput=False)
    emb = nc.declare_dram_parameter("embedding", [K, D], f32, isOutput=False)
    zq = nc.declare_dram_parameter("z_q", [shard, D], f32, isOutput=True)
    # partition-major: idx_pt[p, n*TB + t] = argmin of row n*1024 + p*8 + t
    idx = nc.declare_dram_parameter("idx", [P, nb * TB], u32, isOutput=True)

    z_v = z.rearrange("(n p t) d -> n p t d", p=P, t=TB)        # [NB, 128, TB, D]
    zq_v = zq.rearrange("(n p t) d -> n p t d", p=P, t=TB)
    idx_v = idx.rearrange("p (n t) -> n p t", t=TB)             # [NB, 128, TB]

    with TileContext(nc) as tc, ExitStack() as ctx:
        const = ctx.enter_context(tc.tile_pool(name="const", bufs=1))
        io = ctx.enter_context(tc.tile_pool(name="io", bufs=6))
        work = ctx.enter_context(tc.tile_pool(name="work", bufs=4))
        psum = ctx.enter_context(tc.tile_pool(name="psum", bufs=2, space="PSUM"))

        # ---- constants -------------------------------------------------
        e_sb = const.tile([K, D], f32)
        nc.sync.dma_start(out=e_sb, in_=emb[:, :])

        ident = const.tile([P, P], bf16)
        make_identity(nc, ident)
        ident32 = const.tile([P, P], f32)
        make_identity(nc, ident32)

        # e rows broadcast to all 128 partitions (for the DVE dot path)
        e_bc = []
        for k in range(K):
            row = const.tile([1, D], f32, name=f"e_row{k}")
            nc.sync.dma_start(out=row, in_=emb[k : k + 1, :])
            t = const.tile([P, D], f32, name=f"e_bc{k}")
            nc.gpsimd.partition_broadcast(t, row)
            e_bc.append(t)

        # f_k = e_k - e_0 broadcast (k=1..3), DVE path operand
        f_bc = []
        for k in range(1, K):
            t = const.tile([P, D], f32, name=f"f_bc{k}")
            nc.vector.tensor_sub(t, e_bc[k], e_bc[0])
            f_bc.append(t)

        # ee[:, k] = ||e_k||^2 ; g_rep = (ee_k - ee_0) repeated TB times
        ee = const.tile([P, K], f32)
        scr_act = const.tile([P, D], f32)
        for k in range(K):
            nc.scalar.activation(
                out=scr_act,
                in_=e_bc[k],
                func=mybir.ActivationFunctionType.Square,
                accum_out=ee[:, k : k + 1],
            )
        gg = const.tile([P, K - 1], f32)
        nc.vector.tensor_scalar(
            gg, ee[:, 1:K], ee[:, 0:1], None, op0=mybir.AluOpType.subtract
        )
        g_rep = const.tile([P, (K - 1) * TB], f32)
        for t in range(TB):
            nc.vector.tensor_copy(g_rep[:, 3 * t : 3 * t + 3], gg)
        g_rep_v = g_rep.rearrange("p (t e) -> p t e", e=3)

        # fT chunks for the PE dot path: fT[:, 3c+j-1] = 2*(e_j - e_0)[c*128 + :]
        eT_ps = psum.tile([P, 3 * TB], f32, tag="s_ps")
        for c in range(2):
            nc.tensor.matmul(
                eT_ps[:, 4 * c : 4 * c + 4],
                e_sb[:, 128 * c : 128 * (c + 1)],
                ident32[0:K, 0:K],
                is_transpose=True,
                start=(c == 0),
                stop=(c == 1),
            )
        eT = const.tile([P, 2 * K], f32)
        nc.vector.tensor_copy(eT, eT_ps[:, 0 : 2 * K])
        fT = const.tile([P, 2 * (K - 1)], f32)
        for c in range(2):
            nc.vector.tensor_scalar(
                fT[:, 3 * c : 3 * c + 3],
                eT[:, 4 * c + 1 : 4 * c + 4],
                eT[:, 4 * c : 4 * c + 1],
                None,
                op0=mybir.AluOpType.subtract,
            )
        nc.vector.tensor_scalar_mul(fT, fT, 2.0)

        # split-bf16 codebook for the exact one-hot gather
        e_hi = const.tile([K, D], bf16)
        nc.vector.tensor_copy(e_hi, e_sb)
        e_hi_f = const.tile([K, D], f32)
        nc.vector.tensor_copy(e_hi_f, e_hi)
        e_lo_f = const.tile([K, D], f32)
        nc.vector.tensor_sub(e_lo_f, e_sb, e_hi_f)
        e_lo = const.tile([K, D], bf16)
        nc.vector.tensor_copy(e_lo, e_lo_f)

        # iota row 0..3 along free, repeated TB times (one-hot compare)
        iota_rep = const.tile([P, K * TB], f32)
        for k in range(K):
            nc.vector.memset(
                iota_rep.rearrange("p (t e) -> p t e", e=K)[:, :, k : k + 1],
                float(k),
            )

        # ---- main loop -------------------------------------------------
        for b in range(nb):
            pe_path = pe_every > 0 and (b % pe_every == 0)

            z_b = io.tile([P, TB * D], f32, tag="z_b")
            nc.sync.dma_start(out=z_b, in_=z_v[b])
            zq_b = io.tile([P, TB * D], f32, tag="zq_b")

            h_b = work.tile([P, 8 * TB], f32, tag="h_b")
            nc.vector.memset(h_b, NEG_BIG)
            h_v = h_b.rearrange("p (t e) -> p t e", e=8)
            nc.vector.memset(h_v[:, :, 0:1], 0.0)

            vt = work.tile([P, 8 * TB], u32, tag="vt", bufs=6)
            scr = work.tile([P, D], f32, tag="ttr_scr")
            mx = work.tile([P, 8], f32, tag="mx")

            # --- h_k = 2<z, e_k - e_0> - (ee_k - ee_0), k = 1..3 -------
            if pe_path:
                s_ps = psum.tile([P, 3 * TB], f32, tag="s_ps")
                for t in range(TB):
                    tp_ps = psum.tile([P, D], f32, tag="tp_ps")
                    for c in range(2):
                        nc.tensor.matmul(
                            tp_ps[:, 128 * c : 128 * (c + 1)],
                            z_b[:, t * D + 128 * c : t * D + 128 * (c + 1)],
                            ident32,
                            is_transpose=True,
                            start=(c == 0),
                            stop=(c == 1),
                        )
                    zT = work.tile([P, D], f32, tag="zT", bufs=3)
                    nc.scalar.copy(out=zT, in_=tp_ps)
                    for c in range(2):
                        nc.tensor.matmul(
                            s_ps[:, 3 * t : 3 * t + 3],
                            zT[:, 128 * c : 128 * (c + 1)],
                            fT[:, 3 * c : 3 * c + 3],
                            start=(t == 0 and c == 0),
                            stop=(t == TB - 1 and c == 1),
                            skip_group_check=not (
                                (t == 0 and c == 0) or (t == TB - 1 and c == 1)
                            ),
                        )
                s_v = s_ps.rearrange("p (t e) -> p t e", e=3)
                nc.vector.tensor_tensor(
                    out=h_v[:, :, 1:4],
                    in0=s_v,
                    in1=g_rep_v,
                    op=mybir.AluOpType.subtract,
                )
            else:
                for t in range(TB):
                    zt = z_b[:, t * D : (t + 1) * D]
                    hs = h_b[:, 8 * t : 8 * t + 8]
                    for k in range(1, K):
                        nc.vector.scalar_tensor_tensor(
                            out=scr,
                            in0=zt,
                            scalar=2.0,
                            in1=f_bc[k - 1],
                            op0=mybir.AluOpType.mult,
                            op1=mybir.AluOpType.mult,
                            accum_out=hs[:, k : k + 1],
                        )
                nc.vector.tensor_tensor(
                    out=h_v[:, :, 1:4],
                    in0=h_v[:, :, 1:4],
                    in1=g_rep_v,
                    op=mybir.AluOpType.subtract,
                )

            # --- argmax + one-hot + gather -----------------------------
            for t in range(TB):
                hs = h_b[:, 8 * t : 8 * t + 8]
                nc.vector.max(out=mx, in_=hs)
                nc.vector.max_index(
                    out=vt[:, 8 * t : 8 * t + 8], in_max=mx, in_values=hs
                )
            idxf = work.tile([P, TB], f32, tag="idxf", bufs=4)
            nc.vector.tensor_copy(idxf, vt[:, 0 : 8 * TB : 8])
            O_big = work.tile([P, K * TB], bf16, tag="O_big", bufs=4)
            nc.vector.tensor_tensor(
                out=O_big.rearrange("p (t e) -> p t e", e=K),
                in0=iota_rep.rearrange("p (t e) -> p t e", e=K),
                in1=idxf[:, :].to_broadcast([P, TB, K]),
                op=mybir.AluOpType.is_equal,
            )

            for q in range(TB // 4):
                ot_ps = psum.tile([K, 4 * P], bf16, tag="ot_ps", bufs=3)
                for u in range(4):
                    t = 4 * q + u
                    nc.tensor.matmul(
                        ot_ps[:, P * u : P * (u + 1)],
                        O_big[:, K * t : K * (t + 1)],
                        ident,
                        is_transpose=True,
                        start=(u == 0),
                        stop=(u == 3),
                        skip_group_check=not (u == 0 or u == 3),
                    )
                ot_sb = work.tile([K, 4 * P], bf16, tag="ot_sb")
                nc.scalar.copy(out=ot_sb, in_=ot_ps)

                for u in range(0, 4, 2):
                    zq_ps = psum.tile([P, 2 * D], f32, tag="zq_ps", bufs=3)
                    for v in range(2):
                        lhsT = ot_sb[:, P * (u + v) : P * (u + v + 1)]
                        nc.tensor.matmul(
                            zq_ps[:, v * D : (v + 1) * D],
                            lhsT,
                            e_hi,
                            start=(v == 0),
                            stop=False,
                            skip_group_check=(v != 0),
                        )
                        nc.tensor.matmul(
                            zq_ps[:, v * D : (v + 1) * D],
                            lhsT,
                            e_lo,
                            start=False,
                            stop=(v == 1),
                            skip_group_check=(v != 1),
                        )
                    t0 = 4 * q + u
                    nc.scalar.copy(
                        out=zq_b[:, t0 * D : (t0 + 2) * D], in_=zq_ps
                    )

            nc.gpsimd.dma_start(out=idx_v[b], in_=vt[:, 0 : 8 * TB : 8])
            nc.gpsimd.dma_start(
                out=zq_v[b][:, 0 : TB // 2, :], in_=zq_b[:, 0 : TB * D // 2]
            )
            nc.gpsimd.dma_start(
                out=zq_v[b][:, TB // 2 :, :], in_=zq_b[:, TB * D // 2 :]
            )

    nc.compile()
    return nc


_NC = None


def _get_nc():
    global _NC
    if _NC is None:
        _NC = build_nc()
    return _NC


def kernel(**inputs):
    from concourse.bass_utils import run_bass_kernel_spmd

    z = np.ascontiguousarray(np.asarray(inputs["z"], dtype=np.float32))
    emb = np.ascontiguousarray(np.asarray(inputs["embedding"], dtype=np.float32))
    nc = _get_nc()
    in_maps = [
        {"z": z[i * SHARD : (i + 1) * SHARD], "embedding": emb}
        for i in range(N_CORES)
    ]
    res = run_bass_kernel_spmd(nc, in_maps, list(range(N_CORES))).results
    z_q = np.concatenate([r["z_q"] for r in res], axis=0)
    idx = np.concatenate(
        [
            r["idx"].reshape(P, NB, TB).transpose(1, 0, 2).reshape(-1)
            for r in res
        ],
        axis=0,
    ).astype(np.int32)
    return z_q, idx
